# revision 13
# baseline (speedup 1.0000x reference)
"""Two-layer LSTM generator (B=4096, H=300, T=64) on 8 TRN2 NeuronCores.

Pure data parallelism: batch sharded 8 ways (512/core), LSTM weights
replicated, no cross-core communication. The per-core program is a
Bass/Tile kernel.

Per-core layout (H=300 padded to Hp=320, everything feature-on-partitions,
batch on the free dim):

- State buffer s = 5 SBUF tiles [128, 512] bf16 covering K=640 rows:
    rows 0-319   "slot0": x / h1  (rows 0-299 real, row 300 == 1.0 bias row)
    rows 320-639 "slot1": h0
  Layer 0 writes h0-new into slot1, layer 1 writes h1-new into slot0, so the
  same buffer feeds both layers and the next step (layer 1's weights have
  their K-halves swapped to match).
- Weights per layer: Wcat [640, 1280] bf16; k-row 300 carries the combined
  bias (multiplied by the constant 1.0 kept in s row 300).  m-columns hold
  the 4 gates in order [g, i, f, o], each padded 300->320 with zeros.  The
  g gate comes first so the cell-state chain (t1 = i*g, c = f*c + t1,
  tanh(c)) runs while the later m-tiles (f, o) are still in the matmul
  phase; only h = o*tanh(c) is left after the last matmul group.
- Matmuls: 10 m-tiles x 5 k-chunks of 128, N=512, into 3 PSUM groups
  (4+4+2 banks).  Within a group the k loop is outermost, ordered so that
  k-chunks depending on the freshest h-strips come last — the PE can start
  a layer's ready chunks while the previous layer's tail strips finish.
- Gate layouts in the GATES tensor [128, 5120] (column block j = m-tile j):
  gates starting at even m-offsets (g @ 0, f @ 640) land in the "C layout"
  (row r -> partition r%128, block r//128); gates at odd offsets (i @ 320,
  o @ 960) land 64-partition-shifted ("F layout").  Cell tensors (t1, t2,
  c, tanh_c) use the C layout; elementwise ops split at the breakpoint
  union {64, 128, 192, 256} into 5 row-strips / 3 row-pieces.
- y_t = h1 is DMA'd transposed+bf16 to DRAM ([T, 320, 512] per core); the
  host reassembles/upcasts to the [B, 1, T, H] fp32 output.
"""

import sys

if "/opt/trn_rl_repo" not in sys.path:
    sys.path.insert(0, "/opt/trn_rl_repo")

from contextlib import ExitStack

import numpy as np
import ml_dtypes

import concourse.bacc as bacc
import concourse.mybir as mybir
import concourse.tile as tile

F32 = mybir.dt.float32
BF16 = mybir.dt.bfloat16

H = 300
HP = 320
B = 512            # per-core batch
K = 2 * HP
NK = 5
N_CORES = 8
SIG = mybir.ActivationFunctionType.Sigmoid
TANH = mybir.ActivationFunctionType.Tanh


def build_nc(T, gate_dt=F32, cell_dt=F32, out_dt=BF16):
    nc = bacc.Bacc(None, target_bir_lowering=False)

    w0 = nc.dram_tensor("w0", [K, 1280], BF16, kind="ExternalInput")
    w1 = nc.dram_tensor("w1", [K, 1280], BF16, kind="ExternalInput")
    # layer-1 k-chunk 2 for t=0 only: slot0 rows zeroed (h1_init = 0), bias kept
    w1f = nc.dram_tensor("w1f", [128, 1280], BF16, kind="ExternalInput")
    xz = nc.dram_tensor("xz", [HP, B], BF16, kind="ExternalInput")
    yT = nc.dram_tensor("yT", [T, HP, B], out_dt, kind="ExternalOutput")

    with tile.TileContext(nc) as tc, ExitStack() as ctx:
        persist = ctx.enter_context(tc.tile_pool(name="persist", bufs=1))
        gates_pool = ctx.enter_context(tc.tile_pool(name="gates", bufs=2))
        cell_pool = ctx.enter_context(tc.tile_pool(name="cell", bufs=2))
        psum_pool = ctx.enter_context(
            tc.tile_pool(name="psum", bufs=2, space="PSUM")
        )

        w_sb = [[persist.tile([128, 1280], BF16, name=f"w{l}_{k}", tag=f"w{l}_{k}")
                 for k in range(NK)] for l in range(2)]
        w1f_sb = persist.tile([128, 1280], BF16, name="w1f", tag="w1f")
        s_sb = [persist.tile([128, B], BF16, name=f"s{k}", tag=f"s{k}")
                for k in range(NK)]
        c_sb = [persist.tile([128, 1536], cell_dt, name=f"c{l}", tag=f"c{l}")
                for l in range(2)]

        for l, w in enumerate((w0, w1)):
            for k in range(NK):
                nc.sync.dma_start(out=w_sb[l][k], in_=w[128 * k:128 * (k + 1), :])
        nc.sync.dma_start(out=w1f_sb, in_=w1f[:, :])
        nc.sync.dma_start(out=s_sb[0], in_=xz[0:128, :])
        nc.sync.dma_start(out=s_sb[1], in_=xz[128:256, :])
        nc.sync.dma_start(out=s_sb[2][0:64, :], in_=xz[256:320, :])
        nc.vector.memset(s_sb[2][64:128, :], 0.0)
        nc.vector.memset(s_sb[3], 0.0)
        nc.vector.memset(s_sb[4], 0.0)
        nc.vector.memset(c_sb[0], 0.0)
        nc.vector.memset(c_sb[1], 0.0)

        groups = [(0, 1, 2, 3), (4, 5, 6, 7), (8, 9)]

        def mm_group(pt, taus, kws):
            # k outermost: ready chunks first, fresh-state chunks last
            for ki, (k, wk) in enumerate(kws):
                for j, tau in enumerate(taus):
                    nc.tensor.matmul(
                        pt[:, 512 * j:512 * (j + 1)],
                        wk[:, 128 * tau:128 * (tau + 1)],
                        s_sb[k],
                        start=(ki == 0),
                        stop=(ki == len(kws) - 1),
                    )

        for t in range(T):
            for l in range(2):
                w = w_sb[l]
                if t == 0 and l == 1:
                    kws = [(2, w1f_sb), (3, w[3]), (4, w[4])]
                elif l == 1:
                    # slot0 (h1) is old state, slot1 (h0) is being written:
                    # k2 needs h0 strip0, k3 strips 1-2, k4 strips 3-4
                    kws = [(k, w[k]) for k in (0, 1, 2, 3, 4)]
                else:
                    # slot1 (h0) is old, slot0 (h1) fresh: do k3,k4 first
                    kws = [(k, w[k]) for k in (3, 4, 0, 1, 2)]

                g_sb = gates_pool.tile([128, 5120], gate_dt, name="g", tag="g")
                o_bf = gates_pool.tile([128, 1536], BF16, name="ob", tag="ob")

                # ---- group A: tau0-3 = g gate (tanh) + f head (sigmoid) ----
                pt = psum_pool.tile([128, 2048], F32, name="psA", tag="ps")
                mm_group(pt, groups[0], kws)
                nc.scalar.activation(g_sb[:, 0:1024], pt[:, 0:1024], TANH)
                nc.scalar.activation(g_sb[0:64, 1024:1536], pt[0:64, 1024:1536], TANH)
                nc.scalar.activation(g_sb[64:128, 1024:1536], pt[64:128, 1024:1536], SIG)
                nc.scalar.activation(g_sb[:, 1536:2048], pt[:, 1536:2048], SIG)

                # ---- group B: tau4-7 = f tail, i, o head (all sigmoid) ----
                pt = psum_pool.tile([128, 2048], F32, name="psB", tag="ps")
                mm_group(pt, groups[1], kws)
                nc.scalar.activation(g_sb[:, 2048:2560], pt[:, 0:512], SIG)
                nc.scalar.activation(g_sb[:, 2560:3584], pt[:, 512:1536], SIG)
                nc.scalar.activation(g_sb[0:64, 3584:4096], pt[0:64, 1536:2048], SIG)
                nc.scalar.activation(o_bf[64:128, 0:512], pt[64:128, 1536:2048], SIG)

                # gate views. Hardware rule: a TensorTensor's two inputs must
                # share a base partition (the output may shift), so the
                # multiplied pairs are kept same-parity: g,i in the C layout
                # (strips/pieces at {128,256}), f,o in the F layout ({64,192}).
                g_s = (g_sb[0:64, 0:512], g_sb[64:128, 0:512],
                       g_sb[0:64, 512:1024], g_sb[64:128, 512:1024],
                       g_sb[0:64, 1024:1536])
                i_s = (g_sb[0:64, 2560:3072], g_sb[64:128, 2560:3072],
                       g_sb[0:64, 3072:3584], g_sb[64:128, 3072:3584],
                       g_sb[0:64, 3584:4096])
                f_r1, f_r2a, f_r2b = (g_sb[64:128, 1024:1536],
                                      g_sb[:, 1536:2048], g_sb[:, 2048:2560])
                o_s = (o_bf[64:128, 0:512], o_bf[0:64, 512:1024],
                       o_bf[64:128, 512:1024], o_bf[0:64, 1024:1536],
                       o_bf[64:128, 1024:1536])

                c = c_sb[l]
                t1 = cell_pool.tile([128, 1536], gate_dt, name="t1", tag="t1")
                t2 = cell_pool.tile([128, 1536], gate_dt, name="t2", tag="t2")
                # o and tanh(c) in bf16: the tail h muls hit the DVE 2x mode
                th = cell_pool.tile([128, 1536], BF16, name="th", tag="th")

                def f_strips(x):
                    return (x[64:128, 0:512], x[0:64, 512:1024],
                            x[64:128, 512:1024], x[0:64, 1024:1536],
                            x[64:128, 1024:1536])

                def f_pieces(x):
                    return (x[64:128, 0:512], x[:, 512:1024], x[:, 1024:1536])

                t1_s, t1_p = f_strips(t1), f_pieces(t1)
                c_p, t2_p = f_pieces(c), f_pieces(t2)
                th_p = f_pieces(th)
                th_s = f_strips(th)

                # cell chain, piecewise in F-layout pieces {r<64, 64-191,
                # 192-319}; t1 = i*g reads C-layout pairs, writes F (out
                # shift is legal), so it is split at {64,128,192,256}
                nc.vector.tensor_mul(t1_s[0], i_s[0], g_s[0])
                nc.vector.tensor_mul(t2_p[0], f_r1, c_p[0])
                nc.vector.tensor_add(c_p[0], t2_p[0], t1_p[0])
                nc.scalar.activation(th_p[0], c_p[0], TANH)

                nc.vector.tensor_mul(t1_s[1], i_s[1], g_s[1])
                nc.vector.tensor_mul(t1_s[2], i_s[2], g_s[2])
                nc.vector.tensor_mul(t2_p[1], f_r2a, c_p[1])
                nc.vector.tensor_add(c_p[1], t2_p[1], t1_p[1])
                nc.scalar.activation(th_p[1], c_p[1], TANH)

                nc.vector.tensor_mul(t1_s[3], i_s[3], g_s[3])
                nc.vector.tensor_mul(t1_s[4], i_s[4], g_s[4])
                nc.vector.tensor_mul(t2_p[2], f_r2b, c_p[2])
                nc.vector.tensor_add(c_p[2], t2_p[2], t1_p[2])
                nc.scalar.activation(th_p[2], c_p[2], TANH)

                # ---- group C: tau8-9 = o tail (sigmoid) ----
                pt = psum_pool.tile([128, 1024], F32, name="psC", tag="ps")
                mm_group(pt, groups[2], kws)
                nc.scalar.activation(o_bf[:, 512:1536], pt, SIG)

                # h = o * tanh(c); must follow ALL of this layer's matmuls
                # (it overwrites the state tiles the matmuls read).  Both
                # inputs are F-layout so h0 (64-shifted slot) needs no input
                # split: 3 ops; h1 (slot0) splits at the C-layout bounds.
                if l == 0:
                    nc.vector.tensor_mul(s_sb[2][64:128, :], o_s[0], th_s[0])
                    nc.vector.tensor_mul(s_sb[3][:, :], o_bf[:, 512:1024],
                                         th[:, 512:1024])
                    nc.vector.tensor_mul(s_sb[4][0:108, :],
                                         o_bf[0:108, 1024:1536],
                                         th[0:108, 1024:1536])
                else:
                    outs = (s_sb[0][0:64, :], s_sb[0][64:128, :],
                            s_sb[1][0:64, :], s_sb[1][64:128, :],
                            s_sb[2][0:44, :])
                    for si in range(5):
                        o_ap, th_ap = o_s[si], th_s[si]
                        if si == 4:  # real rows only; keeps the bias row
                            o_ap = o_ap[0:44, :]
                            th_ap = th_ap[0:44, :]
                        nc.vector.tensor_mul(outs[si], o_ap, th_ap)

            nc.sync.dma_start(out=yT[t, 0:128, :], in_=s_sb[0])
            nc.sync.dma_start(out=yT[t, 128:256, :], in_=s_sb[1])
            nc.sync.dma_start(out=yT[t, 256:320, :], in_=s_sb[2][0:64, :])

    return nc


_GATE_ORDER = (2, 1, 0, 3)  # ours [g,f,i,o] -> torch gate indices [i,f,g,o]


def _pack_w(w_x, w_h, b, swap):
    out = np.zeros((K, 1280), np.float32)
    for gi, og in enumerate(_GATE_ORDER):
        rows = slice(og * H, (og + 1) * H)
        cols = slice(gi * HP, gi * HP + H)
        out[0:H, cols] = w_x[rows, :].T
        out[300, cols] = b[rows]
        out[HP:HP + H, cols] = w_h[rows, :].T
    if swap:
        out = np.concatenate([out[HP:], out[:HP]], axis=0)
        out[300], out[HP + 300] = out[HP + 300].copy(), out[300].copy()
    return out.astype(ml_dtypes.bfloat16)


def _prep_shared(W_ih0, W_hh0, b0, W_ih1, W_hh1, b1):
    w0 = _pack_w(W_ih0, W_hh0, b0, swap=False)
    w1 = _pack_w(W_ih1, W_hh1, b1, swap=True)
    # chunk-2 rows 0-63 are slot0 (h1 side, zero at t=0 except bias row 44);
    # rows 64-127 are slot1 (h0 side) and must be kept
    w1f = np.array(w1[256:384], np.float32)
    w1f[0:44] = 0.0
    w1f[45:64] = 0.0
    return w0, w1, w1f.astype(ml_dtypes.bfloat16)


def prep_core_inputs(z_shard, W_ih0, W_hh0, b0, W_ih1, W_hh1, b1):
    """Single-core in_map (used by the dev/sim harnesses)."""
    w0, w1, w1f = _prep_shared(W_ih0, W_hh0, b0, W_ih1, W_hh1, b1)
    xz = np.zeros((HP, B), np.float32)
    xz[0:H, :] = z_shard.T
    xz[300, :] = 1.0
    return {"w0": w0, "w1": w1, "w1f": w1f,
            "xz": xz.astype(ml_dtypes.bfloat16)}


_NC_CACHE = {}
last_results = None


def kernel(z, W_ih0, W_hh0, b_ih0, b_hh0, W_ih1, W_hh1, b_ih1, b_hh1,
           sentence_len):
    global last_results
    from concourse.bass_utils import run_bass_kernel_spmd

    T = int(sentence_len)
    if T not in _NC_CACHE:
        nc = build_nc(T)
        nc.compile()
        _NC_CACHE[T] = nc
    nc = _NC_CACHE[T]

    z = np.asarray(z, np.float32)
    b0 = np.asarray(b_ih0, np.float32) + np.asarray(b_hh0, np.float32)
    b1 = np.asarray(b_ih1, np.float32) + np.asarray(b_hh1, np.float32)
    w0, w1, w1f = _prep_shared(np.asarray(W_ih0, np.float32),
                               np.asarray(W_hh0, np.float32), b0,
                               np.asarray(W_ih1, np.float32),
                               np.asarray(W_hh1, np.float32), b1)

    in_maps = []
    for i in range(N_CORES):
        xz = np.zeros((HP, B), np.float32)
        xz[0:H, :] = z[i * B:(i + 1) * B, :].T
        xz[300, :] = 1.0
        in_maps.append({"w0": w0, "w1": w1, "w1f": w1f,
                        "xz": xz.astype(ml_dtypes.bfloat16)})

    last_results = run_bass_kernel_spmd(
        nc, in_maps, core_ids=list(range(N_CORES)))

    out = np.empty((N_CORES * B, 1, T, H), np.float32)
    for i, r in enumerate(last_results.results):
        yT = np.asarray(r["yT"])  # [T, 320, 512] bf16
        u = yT.view(np.uint16)
        u32 = u[:, 0:H, :].astype(np.uint32) << 16          # [T, H, B]
        out[i * B:(i + 1) * B, 0] = (
            u32.view(np.float32).transpose(2, 0, 1))
    return out


# revision 14
# speedup vs baseline: 1223.7785x; 1223.7785x over previous
"""Two-layer LSTM generator (B=4096, H=300, T=64) on 8 TRN2 NeuronCores.

Pure data parallelism: batch sharded 8 ways (512/core), LSTM weights
replicated, no cross-core communication. The per-core program is a
Bass/Tile kernel.

Per-core layout (H=300 padded to Hp=320, everything feature-on-partitions,
batch on the free dim):

- State buffer s = 5 SBUF tiles [128, 512] bf16 covering K=640 rows:
    rows 0-319   "slot0": x / h1  (rows 0-299 real, row 300 == 1.0 bias row)
    rows 320-639 "slot1": h0
  Layer 0 writes h0-new into slot1, layer 1 writes h1-new into slot0, so the
  same buffer feeds both layers and the next step (layer 1's weights have
  their K-halves swapped to match).
- Weights per layer: Wcat [640, 1280] bf16; k-row 300 carries the combined
  bias (multiplied by the constant 1.0 kept in s row 300).  m-columns hold
  the 4 gates in order [g, i, f, o], each padded 300->320 with zeros.  The
  g gate comes first so the cell-state chain (t1 = i*g, c = f*c + t1,
  tanh(c)) runs while the later m-tiles (f, o) are still in the matmul
  phase; only h = o*tanh(c) is left after the last matmul group.
- Matmuls: 10 m-tiles x 5 k-chunks of 128, N=512, into 3 PSUM groups
  (4+4+2 banks).  Within a group the k loop is outermost, ordered so that
  k-chunks depending on the freshest h-strips come last — the PE can start
  a layer's ready chunks while the previous layer's tail strips finish.
- Gate layouts in the GATES tensor [128, 5120] (column block j = m-tile j):
  gates starting at even m-offsets (g @ 0, f @ 640) land in the "C layout"
  (row r -> partition r%128, block r//128); gates at odd offsets (i @ 320,
  o @ 960) land 64-partition-shifted ("F layout").  Cell tensors (t1, t2,
  c, tanh_c) use the C layout; elementwise ops split at the breakpoint
  union {64, 128, 192, 256} into 5 row-strips / 3 row-pieces.
- y_t = h1 is DMA'd transposed+bf16 to DRAM ([T, 320, 512] per core); the
  host reassembles/upcasts to the [B, 1, T, H] fp32 output.
"""

import sys

if "/opt/trn_rl_repo" not in sys.path:
    sys.path.insert(0, "/opt/trn_rl_repo")

from contextlib import ExitStack

import numpy as np
import ml_dtypes

import concourse.bacc as bacc
import concourse.mybir as mybir
import concourse.tile as tile

F32 = mybir.dt.float32
BF16 = mybir.dt.bfloat16

H = 300
HP = 320
B = 512            # per-core batch
K = 2 * HP
NK = 5
N_CORES = 8
SIG = mybir.ActivationFunctionType.Sigmoid
TANH = mybir.ActivationFunctionType.Tanh


def build_nc(T, gate_dt=F32, cell_dt=F32, out_dt=BF16):
    nc = bacc.Bacc(None, target_bir_lowering=False)

    w0 = nc.dram_tensor("w0", [K, 1280], BF16, kind="ExternalInput")
    w1 = nc.dram_tensor("w1", [K, 1280], BF16, kind="ExternalInput")
    # layer-1 k-chunk 2 for t=0 only: slot0 rows zeroed (h1_init = 0), bias kept
    w1f = nc.dram_tensor("w1f", [128, 1280], BF16, kind="ExternalInput")
    xz = nc.dram_tensor("xz", [HP, B], BF16, kind="ExternalInput")
    yT = nc.dram_tensor("yT", [T, H, B], out_dt, kind="ExternalOutput")

    with tile.TileContext(nc) as tc, ExitStack() as ctx:
        persist = ctx.enter_context(tc.tile_pool(name="persist", bufs=1))
        gates_pool = ctx.enter_context(tc.tile_pool(name="gates", bufs=2))
        cell_pool = ctx.enter_context(tc.tile_pool(name="cell", bufs=2))
        psum_pool = ctx.enter_context(
            tc.tile_pool(name="psum", bufs=2, space="PSUM")
        )

        w_sb = [[persist.tile([128, 1280], BF16, name=f"w{l}_{k}", tag=f"w{l}_{k}")
                 for k in range(NK)] for l in range(2)]
        w1f_sb = persist.tile([128, 1280], BF16, name="w1f", tag="w1f")
        s_sb = [persist.tile([128, B], BF16, name=f"s{k}", tag=f"s{k}")
                for k in range(NK)]
        c_sb = [persist.tile([128, 1536], cell_dt, name=f"c{l}", tag=f"c{l}")
                for l in range(2)]

        for l, w in enumerate((w0, w1)):
            for k in range(NK):
                nc.sync.dma_start(out=w_sb[l][k], in_=w[128 * k:128 * (k + 1), :])
        nc.sync.dma_start(out=w1f_sb, in_=w1f[:, :])
        nc.sync.dma_start(out=s_sb[0], in_=xz[0:128, :])
        nc.sync.dma_start(out=s_sb[1], in_=xz[128:256, :])
        nc.sync.dma_start(out=s_sb[2][0:64, :], in_=xz[256:320, :])
        nc.vector.memset(s_sb[2][64:128, :], 0.0)
        nc.vector.memset(s_sb[3], 0.0)
        nc.vector.memset(s_sb[4], 0.0)
        nc.vector.memset(c_sb[0], 0.0)
        nc.vector.memset(c_sb[1], 0.0)

        groups = [(0, 1, 2, 3), (4, 5, 6, 7), (8, 9)]

        def mm_group(pt, taus, kws):
            # k outermost: ready chunks first, fresh-state chunks last
            for ki, (k, wk) in enumerate(kws):
                for j, tau in enumerate(taus):
                    nc.tensor.matmul(
                        pt[:, 512 * j:512 * (j + 1)],
                        wk[:, 128 * tau:128 * (tau + 1)],
                        s_sb[k],
                        start=(ki == 0),
                        stop=(ki == len(kws) - 1),
                    )

        for t in range(T):
            for l in range(2):
                w = w_sb[l]
                if t == 0 and l == 1:
                    kws = [(2, w1f_sb), (3, w[3]), (4, w[4])]
                elif l == 1:
                    # slot0 (h1) is old state, slot1 (h0) is being written:
                    # k2 needs h0 strip0, k3 strips 1-2, k4 strips 3-4
                    kws = [(k, w[k]) for k in (0, 1, 2, 3, 4)]
                else:
                    # slot1 (h0) is old, slot0 (h1) fresh: do k3,k4 first
                    kws = [(k, w[k]) for k in (3, 4, 0, 1, 2)]

                g_sb = gates_pool.tile([128, 5120], gate_dt, name="g", tag="g")
                o_bf = gates_pool.tile([128, 1536], BF16, name="ob", tag="ob")

                # ---- group A: tau0-3 = g gate (tanh) + f head (sigmoid) ----
                pt = psum_pool.tile([128, 2048], F32, name="psA", tag="ps")
                mm_group(pt, groups[0], kws)
                nc.scalar.activation(g_sb[:, 0:1024], pt[:, 0:1024], TANH)
                nc.scalar.activation(g_sb[0:64, 1024:1536], pt[0:64, 1024:1536], TANH)
                nc.scalar.activation(g_sb[64:128, 1024:1536], pt[64:128, 1024:1536], SIG)
                nc.scalar.activation(g_sb[:, 1536:2048], pt[:, 1536:2048], SIG)

                # ---- group B: tau4-7 = f tail, i, o head (all sigmoid) ----
                pt = psum_pool.tile([128, 2048], F32, name="psB", tag="ps")
                mm_group(pt, groups[1], kws)
                nc.scalar.activation(g_sb[:, 2048:2560], pt[:, 0:512], SIG)
                nc.scalar.activation(g_sb[:, 2560:3584], pt[:, 512:1536], SIG)
                nc.scalar.activation(g_sb[0:64, 3584:4096], pt[0:64, 1536:2048], SIG)
                nc.scalar.activation(o_bf[64:128, 0:512], pt[64:128, 1536:2048], SIG)

                # gate views. Hardware rule: a TensorTensor's two inputs must
                # share a base partition (the output may shift), so the
                # multiplied pairs are kept same-parity: g,i in the C layout
                # (strips/pieces at {128,256}), f,o in the F layout ({64,192}).
                g_s = (g_sb[0:64, 0:512], g_sb[64:128, 0:512],
                       g_sb[0:64, 512:1024], g_sb[64:128, 512:1024],
                       g_sb[0:64, 1024:1536])
                i_s = (g_sb[0:64, 2560:3072], g_sb[64:128, 2560:3072],
                       g_sb[0:64, 3072:3584], g_sb[64:128, 3072:3584],
                       g_sb[0:64, 3584:4096])
                f_r1, f_r2a, f_r2b = (g_sb[64:128, 1024:1536],
                                      g_sb[:, 1536:2048], g_sb[:, 2048:2560])
                o_s = (o_bf[64:128, 0:512], o_bf[0:64, 512:1024],
                       o_bf[64:128, 512:1024], o_bf[0:64, 1024:1536],
                       o_bf[64:128, 1024:1536])

                c = c_sb[l]
                t1 = cell_pool.tile([128, 1536], gate_dt, name="t1", tag="t1")
                t2 = cell_pool.tile([128, 1536], gate_dt, name="t2", tag="t2")
                # o and tanh(c) in bf16: the tail h muls hit the DVE 2x mode
                th = cell_pool.tile([128, 1536], BF16, name="th", tag="th")

                def f_strips(x):
                    return (x[64:128, 0:512], x[0:64, 512:1024],
                            x[64:128, 512:1024], x[0:64, 1024:1536],
                            x[64:128, 1024:1536])

                def f_pieces(x):
                    return (x[64:128, 0:512], x[:, 512:1024], x[:, 1024:1536])

                t1_s, t1_p = f_strips(t1), f_pieces(t1)
                c_p, t2_p = f_pieces(c), f_pieces(t2)
                th_p = f_pieces(th)
                th_s = f_strips(th)

                # cell chain, piecewise in F-layout pieces {r<64, 64-191,
                # 192-319}; t1 = i*g reads C-layout pairs, writes F (out
                # shift is legal), so it is split at {64,128,192,256}
                nc.vector.tensor_mul(t1_s[0], i_s[0], g_s[0])
                nc.vector.tensor_mul(t2_p[0], f_r1, c_p[0])
                nc.vector.tensor_add(c_p[0], t2_p[0], t1_p[0])
                nc.scalar.activation(th_p[0], c_p[0], TANH)

                nc.vector.tensor_mul(t1_s[1], i_s[1], g_s[1])
                nc.vector.tensor_mul(t1_s[2], i_s[2], g_s[2])
                nc.vector.tensor_mul(t2_p[1], f_r2a, c_p[1])
                nc.vector.tensor_add(c_p[1], t2_p[1], t1_p[1])
                nc.scalar.activation(th_p[1], c_p[1], TANH)

                nc.vector.tensor_mul(t1_s[3], i_s[3], g_s[3])
                nc.vector.tensor_mul(t1_s[4], i_s[4], g_s[4])
                nc.vector.tensor_mul(t2_p[2], f_r2b, c_p[2])
                nc.vector.tensor_add(c_p[2], t2_p[2], t1_p[2])
                nc.scalar.activation(th_p[2], c_p[2], TANH)

                # ---- group C: tau8-9 = o tail (sigmoid) ----
                pt = psum_pool.tile([128, 1024], F32, name="psC", tag="ps")
                mm_group(pt, groups[2], kws)
                nc.scalar.activation(o_bf[:, 512:1536], pt, SIG)

                # h = o * tanh(c); must follow ALL of this layer's matmuls
                # (it overwrites the state tiles the matmuls read).  Both
                # inputs are F-layout so h0 (64-shifted slot) needs no input
                # split: 3 ops; h1 (slot0) splits at the C-layout bounds.
                if l == 0:
                    nc.vector.tensor_mul(s_sb[2][64:128, :], o_s[0], th_s[0])
                    nc.vector.tensor_mul(s_sb[3][:, :], o_bf[:, 512:1024],
                                         th[:, 512:1024])
                    nc.vector.tensor_mul(s_sb[4][0:108, :],
                                         o_bf[0:108, 1024:1536],
                                         th[0:108, 1024:1536])
                else:
                    outs = (s_sb[0][0:64, :], s_sb[0][64:128, :],
                            s_sb[1][0:64, :], s_sb[1][64:128, :],
                            s_sb[2][0:44, :])
                    for si in range(5):
                        o_ap, th_ap = o_s[si], th_s[si]
                        if si == 4:  # real rows only; keeps the bias row
                            o_ap = o_ap[0:44, :]
                            th_ap = th_ap[0:44, :]
                        nc.vector.tensor_mul(outs[si], o_ap, th_ap)

            nc.sync.dma_start(out=yT[t, 0:128, :], in_=s_sb[0])
            nc.sync.dma_start(out=yT[t, 128:256, :], in_=s_sb[1])
            nc.sync.dma_start(out=yT[t, 256:300, :], in_=s_sb[2][0:44, :])

    return nc


_GATE_ORDER = (2, 1, 0, 3)  # ours [g,f,i,o] -> torch gate indices [i,f,g,o]


def _pack_w(w_x, w_h, b, swap):
    out = np.zeros((K, 1280), np.float32)
    for gi, og in enumerate(_GATE_ORDER):
        rows = slice(og * H, (og + 1) * H)
        cols = slice(gi * HP, gi * HP + H)
        out[0:H, cols] = w_x[rows, :].T
        out[300, cols] = b[rows]
        out[HP:HP + H, cols] = w_h[rows, :].T
    if swap:
        out = np.concatenate([out[HP:], out[:HP]], axis=0)
        out[300], out[HP + 300] = out[HP + 300].copy(), out[300].copy()
    return out.astype(ml_dtypes.bfloat16)


def _prep_shared(W_ih0, W_hh0, b0, W_ih1, W_hh1, b1):
    w0 = _pack_w(W_ih0, W_hh0, b0, swap=False)
    w1 = _pack_w(W_ih1, W_hh1, b1, swap=True)
    # chunk-2 rows 0-63 are slot0 (h1 side, zero at t=0 except bias row 44);
    # rows 64-127 are slot1 (h0 side) and must be kept
    w1f = np.array(w1[256:384], np.float32)
    w1f[0:44] = 0.0
    w1f[45:64] = 0.0
    return w0, w1, w1f.astype(ml_dtypes.bfloat16)


def prep_core_inputs(z_shard, W_ih0, W_hh0, b0, W_ih1, W_hh1, b1):
    """Single-core in_map (used by the dev/sim harnesses)."""
    w0, w1, w1f = _prep_shared(W_ih0, W_hh0, b0, W_ih1, W_hh1, b1)
    xz = np.zeros((HP, B), np.float32)
    xz[0:H, :] = z_shard.T
    xz[300, :] = 1.0
    return {"w0": w0, "w1": w1, "w1f": w1f,
            "xz": xz.astype(ml_dtypes.bfloat16)}


_NC_CACHE = {}
last_results = None


def kernel(z, W_ih0, W_hh0, b_ih0, b_hh0, W_ih1, W_hh1, b_ih1, b_hh1,
           sentence_len):
    global last_results
    from concourse.bass_utils import run_bass_kernel_spmd

    T = int(sentence_len)
    if T not in _NC_CACHE:
        nc = build_nc(T)
        nc.compile()
        _NC_CACHE[T] = nc
    nc = _NC_CACHE[T]

    z = np.asarray(z, np.float32)
    b0 = np.asarray(b_ih0, np.float32) + np.asarray(b_hh0, np.float32)
    b1 = np.asarray(b_ih1, np.float32) + np.asarray(b_hh1, np.float32)
    w0, w1, w1f = _prep_shared(np.asarray(W_ih0, np.float32),
                               np.asarray(W_hh0, np.float32), b0,
                               np.asarray(W_ih1, np.float32),
                               np.asarray(W_hh1, np.float32), b1)

    in_maps = []
    for i in range(N_CORES):
        xz = np.zeros((HP, B), np.float32)
        xz[0:H, :] = z[i * B:(i + 1) * B, :].T
        xz[300, :] = 1.0
        in_maps.append({"w0": w0, "w1": w1, "w1f": w1f,
                        "xz": xz.astype(ml_dtypes.bfloat16)})

    last_results = run_bass_kernel_spmd(
        nc, in_maps, core_ids=list(range(N_CORES)))

    out = np.empty((N_CORES * B, 1, T, H), np.float32)
    for i, r in enumerate(last_results.results):
        yT = np.asarray(r["yT"])  # [T, 300, 512] bf16
        u32 = yT.view(np.uint16).astype(np.uint32) << 16    # [T, H, B]
        out[i * B:(i + 1) * B, 0] = (
            u32.view(np.float32).transpose(2, 0, 1))
    return out


# revision 15
# speedup vs baseline: 5351.2473x; 4.3727x over previous
"""Two-layer LSTM generator (B=4096, H=300, T=64) on 8 TRN2 NeuronCores.

Pure data parallelism: batch sharded 8 ways (512/core), LSTM weights
replicated, no cross-core communication. The per-core program is a
Bass/Tile kernel.

Per-core layout (H=300 padded to Hp=320, everything feature-on-partitions,
batch on the free dim):

- State buffer s = 5 SBUF tiles [128, 512] bf16 covering K=640 rows:
    rows 0-319   "slot0": x / h1  (rows 0-299 real, row 300 == 1.0 bias row)
    rows 320-639 "slot1": h0
  Layer 0 writes h0-new into slot1, layer 1 writes h1-new into slot0, so the
  same buffer feeds both layers and the next step (layer 1's weights have
  their K-halves swapped to match).
- Weights per layer: Wcat [640, 1280] bf16; k-row 300 carries the combined
  bias (multiplied by the constant 1.0 kept in s row 300).  m-columns hold
  the 4 gates in order [g, i, f, o], each padded 300->320 with zeros.  The
  g gate comes first so the cell-state chain (t1 = i*g, c = f*c + t1,
  tanh(c)) runs while the later m-tiles (f, o) are still in the matmul
  phase; only h = o*tanh(c) is left after the last matmul group.
- Matmuls: 10 m-tiles x 5 k-chunks of 128, N=512, into 3 PSUM groups
  (4+4+2 banks).  Within a group the k loop is outermost, ordered so that
  k-chunks depending on the freshest h-strips come last — the PE can start
  a layer's ready chunks while the previous layer's tail strips finish.
- Gate layouts in the GATES tensor [128, 5120] (column block j = m-tile j):
  gates starting at even m-offsets (g @ 0, f @ 640) land in the "C layout"
  (row r -> partition r%128, block r//128); gates at odd offsets (i @ 320,
  o @ 960) land 64-partition-shifted ("F layout").  Cell tensors (t1, t2,
  c, tanh_c) use the C layout; elementwise ops split at the breakpoint
  union {64, 128, 192, 256} into 5 row-strips / 3 row-pieces.
- y_t = h1 is DMA'd transposed+bf16 to DRAM ([T, 320, 512] per core); the
  host reassembles/upcasts to the [B, 1, T, H] fp32 output.
"""

import sys

if "/opt/trn_rl_repo" not in sys.path:
    sys.path.insert(0, "/opt/trn_rl_repo")

from contextlib import ExitStack

import numpy as np
import ml_dtypes

import concourse.bacc as bacc
import concourse.mybir as mybir
import concourse.tile as tile

F32 = mybir.dt.float32
BF16 = mybir.dt.bfloat16

H = 300
HP = 320
B = 512            # per-core batch
K = 2 * HP
NK = 5
N_CORES = 8
SIG = mybir.ActivationFunctionType.Sigmoid
TANH = mybir.ActivationFunctionType.Tanh


def build_nc(T, gate_dt=F32, cell_dt=F32, out_dt=BF16):
    nc = bacc.Bacc(None, target_bir_lowering=False)

    w0 = nc.dram_tensor("w0", [K, 1280], BF16, kind="ExternalInput")
    w1 = nc.dram_tensor("w1", [K, 1280], BF16, kind="ExternalInput")
    # layer-1 k-chunk 2 for t=0 only: slot0 rows zeroed (h1_init = 0), bias kept
    w1f = nc.dram_tensor("w1f", [128, 1280], BF16, kind="ExternalInput")
    xz = nc.dram_tensor("xz", [HP, B], BF16, kind="ExternalInput")
    yT = nc.dram_tensor("yT", [T, H, B], out_dt, kind="ExternalOutput")

    with tile.TileContext(nc) as tc, ExitStack() as ctx:
        persist = ctx.enter_context(tc.tile_pool(name="persist", bufs=1))
        gates_pool = ctx.enter_context(tc.tile_pool(name="gates", bufs=3))
        cell_pool = ctx.enter_context(tc.tile_pool(name="cell", bufs=3))
        psum_pool = ctx.enter_context(
            tc.tile_pool(name="psum", bufs=2, space="PSUM")
        )

        w_sb = [[persist.tile([128, 1280], BF16, name=f"w{l}_{k}", tag=f"w{l}_{k}")
                 for k in range(NK)] for l in range(2)]
        w1f_sb = persist.tile([128, 1280], BF16, name="w1f", tag="w1f")
        s_sb = [persist.tile([128, B], BF16, name=f"s{k}", tag=f"s{k}")
                for k in range(NK)]
        c_sb = [persist.tile([128, 1536], cell_dt, name=f"c{l}", tag=f"c{l}")
                for l in range(2)]

        for l, w in enumerate((w0, w1)):
            for k in range(NK):
                nc.sync.dma_start(out=w_sb[l][k], in_=w[128 * k:128 * (k + 1), :])
        nc.sync.dma_start(out=w1f_sb, in_=w1f[:, :])
        nc.sync.dma_start(out=s_sb[0], in_=xz[0:128, :])
        nc.sync.dma_start(out=s_sb[1], in_=xz[128:256, :])
        nc.sync.dma_start(out=s_sb[2][0:64, :], in_=xz[256:320, :])
        nc.vector.memset(s_sb[2][64:128, :], 0.0)
        nc.vector.memset(s_sb[3], 0.0)
        nc.vector.memset(s_sb[4], 0.0)
        nc.vector.memset(c_sb[0], 0.0)
        nc.vector.memset(c_sb[1], 0.0)

        groups = [(0, 1, 2, 3), (4, 5, 6, 7), (8, 9)]

        def mm_group(pt, taus, kws):
            # k outermost: ready chunks first, fresh-state chunks last
            for ki, (k, wk) in enumerate(kws):
                for j, tau in enumerate(taus):
                    nc.tensor.matmul(
                        pt[:, 512 * j:512 * (j + 1)],
                        wk[:, 128 * tau:128 * (tau + 1)],
                        s_sb[k],
                        start=(ki == 0),
                        stop=(ki == len(kws) - 1),
                    )

        for t in range(T):
            for l in range(2):
                w = w_sb[l]
                if t == 0 and l == 1:
                    kws = [(2, w1f_sb), (3, w[3]), (4, w[4])]
                elif l == 1:
                    # slot0 (h1) is old state, slot1 (h0) is being written:
                    # k2 needs h0 strip0, k3 strips 1-2, k4 strips 3-4
                    kws = [(k, w[k]) for k in (0, 1, 2, 3, 4)]
                else:
                    # slot1 (h0) is old, slot0 (h1) fresh: do k3,k4 first
                    kws = [(k, w[k]) for k in (3, 4, 0, 1, 2)]

                g_sb = gates_pool.tile([128, 5120], gate_dt, name="g", tag="g")
                o_bf = gates_pool.tile([128, 1536], BF16, name="ob", tag="ob")

                # ---- group A: tau0-3 = g gate (tanh) + f head (sigmoid) ----
                pt = psum_pool.tile([128, 2048], F32, name="psA", tag="ps")
                mm_group(pt, groups[0], kws)
                nc.scalar.activation(g_sb[:, 0:1024], pt[:, 0:1024], TANH)
                nc.scalar.activation(g_sb[0:64, 1024:1536], pt[0:64, 1024:1536], TANH)
                nc.scalar.activation(g_sb[64:128, 1024:1536], pt[64:128, 1024:1536], SIG)
                nc.scalar.activation(g_sb[:, 1536:2048], pt[:, 1536:2048], SIG)

                # ---- group B: tau4-7 = f tail, i, o head (all sigmoid) ----
                pt = psum_pool.tile([128, 2048], F32, name="psB", tag="ps")
                mm_group(pt, groups[1], kws)
                nc.scalar.activation(g_sb[:, 2048:2560], pt[:, 0:512], SIG)
                nc.scalar.activation(g_sb[:, 2560:3584], pt[:, 512:1536], SIG)
                nc.scalar.activation(g_sb[0:64, 3584:4096], pt[0:64, 1536:2048], SIG)
                nc.scalar.activation(o_bf[64:128, 0:512], pt[64:128, 1536:2048], SIG)

                # gate views. Hardware rule: a TensorTensor's two inputs must
                # share a base partition (the output may shift), so the
                # multiplied pairs are kept same-parity: g,i in the C layout
                # (strips/pieces at {128,256}), f,o in the F layout ({64,192}).
                g_s = (g_sb[0:64, 0:512], g_sb[64:128, 0:512],
                       g_sb[0:64, 512:1024], g_sb[64:128, 512:1024],
                       g_sb[0:64, 1024:1536])
                i_s = (g_sb[0:64, 2560:3072], g_sb[64:128, 2560:3072],
                       g_sb[0:64, 3072:3584], g_sb[64:128, 3072:3584],
                       g_sb[0:64, 3584:4096])
                f_r1, f_r2a, f_r2b = (g_sb[64:128, 1024:1536],
                                      g_sb[:, 1536:2048], g_sb[:, 2048:2560])
                o_s = (o_bf[64:128, 0:512], o_bf[0:64, 512:1024],
                       o_bf[64:128, 512:1024], o_bf[0:64, 1024:1536],
                       o_bf[64:128, 1024:1536])

                c = c_sb[l]
                t1 = cell_pool.tile([128, 1536], gate_dt, name="t1", tag="t1")
                t2 = cell_pool.tile([128, 1536], gate_dt, name="t2", tag="t2")
                # o and tanh(c) in bf16: the tail h muls hit the DVE 2x mode
                th = cell_pool.tile([128, 1536], BF16, name="th", tag="th")

                def f_strips(x):
                    return (x[64:128, 0:512], x[0:64, 512:1024],
                            x[64:128, 512:1024], x[0:64, 1024:1536],
                            x[64:128, 1024:1536])

                def f_pieces(x):
                    return (x[64:128, 0:512], x[:, 512:1024], x[:, 1024:1536])

                t1_s, t1_p = f_strips(t1), f_pieces(t1)
                c_p, t2_p = f_pieces(c), f_pieces(t2)
                th_p = f_pieces(th)
                th_s = f_strips(th)

                # cell chain, piecewise in F-layout pieces {r<64, 64-191,
                # 192-319}; t1 = i*g reads C-layout pairs, writes F (out
                # shift is legal), so it is split at {64,128,192,256}
                nc.vector.tensor_mul(t1_s[0], i_s[0], g_s[0])
                nc.vector.tensor_mul(t2_p[0], f_r1, c_p[0])
                nc.vector.tensor_add(c_p[0], t2_p[0], t1_p[0])
                nc.scalar.activation(th_p[0], c_p[0], TANH)

                nc.vector.tensor_mul(t1_s[1], i_s[1], g_s[1])
                nc.vector.tensor_mul(t1_s[2], i_s[2], g_s[2])
                nc.vector.tensor_mul(t2_p[1], f_r2a, c_p[1])
                nc.vector.tensor_add(c_p[1], t2_p[1], t1_p[1])
                nc.scalar.activation(th_p[1], c_p[1], TANH)

                nc.vector.tensor_mul(t1_s[3], i_s[3], g_s[3])
                nc.vector.tensor_mul(t1_s[4], i_s[4], g_s[4])
                nc.vector.tensor_mul(t2_p[2], f_r2b, c_p[2])
                nc.vector.tensor_add(c_p[2], t2_p[2], t1_p[2])
                nc.scalar.activation(th_p[2], c_p[2], TANH)

                # ---- group C: tau8-9 = o tail (sigmoid) ----
                pt = psum_pool.tile([128, 1024], F32, name="psC", tag="ps")
                mm_group(pt, groups[2], kws)
                nc.scalar.activation(o_bf[:, 512:1536], pt, SIG)

                # h = o * tanh(c); must follow ALL of this layer's matmuls
                # (it overwrites the state tiles the matmuls read).  Both
                # inputs are F-layout so h0 (64-shifted slot) needs no input
                # split: 3 ops; h1 (slot0) splits at the C-layout bounds.
                if l == 0:
                    nc.vector.tensor_mul(s_sb[2][64:128, :], o_s[0], th_s[0])
                    nc.vector.tensor_mul(s_sb[3][:, :], o_bf[:, 512:1024],
                                         th[:, 512:1024])
                    nc.vector.tensor_mul(s_sb[4][0:108, :],
                                         o_bf[0:108, 1024:1536],
                                         th[0:108, 1024:1536])
                else:
                    outs = (s_sb[0][0:64, :], s_sb[0][64:128, :],
                            s_sb[1][0:64, :], s_sb[1][64:128, :],
                            s_sb[2][0:44, :])
                    for si in range(5):
                        o_ap, th_ap = o_s[si], th_s[si]
                        if si == 4:  # real rows only; keeps the bias row
                            o_ap = o_ap[0:44, :]
                            th_ap = th_ap[0:44, :]
                        nc.vector.tensor_mul(outs[si], o_ap, th_ap)

            nc.sync.dma_start(out=yT[t, 0:128, :], in_=s_sb[0])
            nc.sync.dma_start(out=yT[t, 128:256, :], in_=s_sb[1])
            nc.sync.dma_start(out=yT[t, 256:300, :], in_=s_sb[2][0:44, :])

    return nc


_GATE_ORDER = (2, 1, 0, 3)  # ours [g,f,i,o] -> torch gate indices [i,f,g,o]


def _pack_w(w_x, w_h, b, swap):
    out = np.zeros((K, 1280), np.float32)
    for gi, og in enumerate(_GATE_ORDER):
        rows = slice(og * H, (og + 1) * H)
        cols = slice(gi * HP, gi * HP + H)
        out[0:H, cols] = w_x[rows, :].T
        out[300, cols] = b[rows]
        out[HP:HP + H, cols] = w_h[rows, :].T
    if swap:
        out = np.concatenate([out[HP:], out[:HP]], axis=0)
        out[300], out[HP + 300] = out[HP + 300].copy(), out[300].copy()
    return out.astype(ml_dtypes.bfloat16)


def _prep_shared(W_ih0, W_hh0, b0, W_ih1, W_hh1, b1):
    w0 = _pack_w(W_ih0, W_hh0, b0, swap=False)
    w1 = _pack_w(W_ih1, W_hh1, b1, swap=True)
    # chunk-2 rows 0-63 are slot0 (h1 side, zero at t=0 except bias row 44);
    # rows 64-127 are slot1 (h0 side) and must be kept
    w1f = np.array(w1[256:384], np.float32)
    w1f[0:44] = 0.0
    w1f[45:64] = 0.0
    return w0, w1, w1f.astype(ml_dtypes.bfloat16)


def prep_core_inputs(z_shard, W_ih0, W_hh0, b0, W_ih1, W_hh1, b1):
    """Single-core in_map (used by the dev/sim harnesses)."""
    w0, w1, w1f = _prep_shared(W_ih0, W_hh0, b0, W_ih1, W_hh1, b1)
    xz = np.zeros((HP, B), np.float32)
    xz[0:H, :] = z_shard.T
    xz[300, :] = 1.0
    return {"w0": w0, "w1": w1, "w1f": w1f,
            "xz": xz.astype(ml_dtypes.bfloat16)}


_NC_CACHE = {}
last_results = None


def kernel(z, W_ih0, W_hh0, b_ih0, b_hh0, W_ih1, W_hh1, b_ih1, b_hh1,
           sentence_len):
    global last_results
    from concourse.bass_utils import run_bass_kernel_spmd

    T = int(sentence_len)
    if T not in _NC_CACHE:
        nc = build_nc(T)
        nc.compile()
        _NC_CACHE[T] = nc
    nc = _NC_CACHE[T]

    z = np.asarray(z, np.float32)
    b0 = np.asarray(b_ih0, np.float32) + np.asarray(b_hh0, np.float32)
    b1 = np.asarray(b_ih1, np.float32) + np.asarray(b_hh1, np.float32)
    w0, w1, w1f = _prep_shared(np.asarray(W_ih0, np.float32),
                               np.asarray(W_hh0, np.float32), b0,
                               np.asarray(W_ih1, np.float32),
                               np.asarray(W_hh1, np.float32), b1)

    in_maps = []
    for i in range(N_CORES):
        xz = np.zeros((HP, B), np.float32)
        xz[0:H, :] = z[i * B:(i + 1) * B, :].T
        xz[300, :] = 1.0
        in_maps.append({"w0": w0, "w1": w1, "w1f": w1f,
                        "xz": xz.astype(ml_dtypes.bfloat16)})

    last_results = run_bass_kernel_spmd(
        nc, in_maps, core_ids=list(range(N_CORES)))

    out = np.empty((N_CORES * B, 1, T, H), np.float32)
    for i, r in enumerate(last_results.results):
        yT = np.asarray(r["yT"])  # [T, 300, 512] bf16
        u32 = yT.view(np.uint16).astype(np.uint32) << 16    # [T, H, B]
        out[i * B:(i + 1) * B, 0] = (
            u32.view(np.float32).transpose(2, 0, 1))
    return out


# revision 16
# speedup vs baseline: 5384.6792x; 1.0062x over previous
"""Two-layer LSTM generator (B=4096, H=300, T=64) on 8 TRN2 NeuronCores.

Pure data parallelism: batch sharded 8 ways (512/core), LSTM weights
replicated, no cross-core communication. The per-core program is a
Bass/Tile kernel.

Per-core layout (H=300 padded to Hp=320, everything feature-on-partitions,
batch on the free dim):

- State buffer s = 5 SBUF tiles [128, 512] bf16 covering K=640 rows:
    rows 0-319   "slot0": x / h1  (rows 0-299 real, row 300 == 1.0 bias row)
    rows 320-639 "slot1": h0
  Layer 0 writes h0-new into slot1, layer 1 writes h1-new into slot0, so the
  same buffer feeds both layers and the next step (layer 1's weights have
  their K-halves swapped to match).
- Weights per layer: Wcat [640, 1280] bf16; k-row 300 carries the combined
  bias (multiplied by the constant 1.0 kept in s row 300).  m-columns hold
  the 4 gates in order [g, i, f, o], each padded 300->320 with zeros.  The
  g gate comes first so the cell-state chain (t1 = i*g, c = f*c + t1,
  tanh(c)) runs while the later m-tiles (f, o) are still in the matmul
  phase; only h = o*tanh(c) is left after the last matmul group.
- Matmuls: 10 m-tiles x 5 k-chunks of 128, N=512, into 3 PSUM groups
  (4+4+2 banks).  Within a group the k loop is outermost, ordered so that
  k-chunks depending on the freshest h-strips come last — the PE can start
  a layer's ready chunks while the previous layer's tail strips finish.
- Gate layouts in the GATES tensor [128, 5120] (column block j = m-tile j):
  gates starting at even m-offsets (g @ 0, f @ 640) land in the "C layout"
  (row r -> partition r%128, block r//128); gates at odd offsets (i @ 320,
  o @ 960) land 64-partition-shifted ("F layout").  Cell tensors (t1, t2,
  c, tanh_c) use the C layout; elementwise ops split at the breakpoint
  union {64, 128, 192, 256} into 5 row-strips / 3 row-pieces.
- y_t = h1 is DMA'd transposed+bf16 to DRAM ([T, 320, 512] per core); the
  host reassembles/upcasts to the [B, 1, T, H] fp32 output.
"""

import sys

if "/opt/trn_rl_repo" not in sys.path:
    sys.path.insert(0, "/opt/trn_rl_repo")

from contextlib import ExitStack

import numpy as np
import ml_dtypes

import concourse.bacc as bacc
import concourse.mybir as mybir
import concourse.tile as tile

F32 = mybir.dt.float32
BF16 = mybir.dt.bfloat16

H = 300
HP = 320
B = 512            # per-core batch
K = 2 * HP
NK = 5
N_CORES = 8
SIG = mybir.ActivationFunctionType.Sigmoid
TANH = mybir.ActivationFunctionType.Tanh


def build_nc(T, gate_dt=F32, cell_dt=F32, out_dt=BF16):
    nc = bacc.Bacc(None, target_bir_lowering=False)

    w0 = nc.dram_tensor("w0", [K, 1280], BF16, kind="ExternalInput")
    w1 = nc.dram_tensor("w1", [K, 1280], BF16, kind="ExternalInput")
    # layer-1 k-chunk 2 for t=0 only: slot0 rows zeroed (h1_init = 0), bias kept
    w1f = nc.dram_tensor("w1f", [128, 1280], BF16, kind="ExternalInput")
    xz = nc.dram_tensor("xz", [HP, B], BF16, kind="ExternalInput")
    yT = nc.dram_tensor("yT", [T, H, B], out_dt, kind="ExternalOutput")

    with tile.TileContext(nc) as tc, ExitStack() as ctx:
        persist = ctx.enter_context(tc.tile_pool(name="persist", bufs=1))
        gates_pool = ctx.enter_context(tc.tile_pool(name="gates", bufs=3))
        cell_pool = ctx.enter_context(tc.tile_pool(name="cell", bufs=3))
        psum_pool = ctx.enter_context(
            tc.tile_pool(name="psum", bufs=2, space="PSUM")
        )

        w_sb = [[persist.tile([128, 1280], BF16, name=f"w{l}_{k}", tag=f"w{l}_{k}")
                 for k in range(NK)] for l in range(2)]
        w1f_sb = persist.tile([128, 1280], BF16, name="w1f", tag="w1f")
        s_sb = [persist.tile([128, B], BF16, name=f"s{k}", tag=f"s{k}")
                for k in range(NK)]
        c_sb = [persist.tile([128, 1536], cell_dt, name=f"c{l}", tag=f"c{l}")
                for l in range(2)]

        for l, w in enumerate((w0, w1)):
            for k in range(NK):
                nc.sync.dma_start(out=w_sb[l][k], in_=w[128 * k:128 * (k + 1), :])
        nc.sync.dma_start(out=w1f_sb, in_=w1f[:, :])
        nc.sync.dma_start(out=s_sb[0], in_=xz[0:128, :])
        nc.sync.dma_start(out=s_sb[1], in_=xz[128:256, :])
        nc.sync.dma_start(out=s_sb[2][0:64, :], in_=xz[256:320, :])
        nc.vector.memset(s_sb[2][64:128, :], 0.0)
        nc.vector.memset(s_sb[3], 0.0)
        nc.vector.memset(s_sb[4], 0.0)
        nc.vector.memset(c_sb[0], 0.0)
        nc.vector.memset(c_sb[1], 0.0)

        groups = [(0, 1, 2, 3), (4, 5, 6, 7), (8, 9)]

        def mm_group(pt, taus, kws):
            # k outermost: ready chunks first, fresh-state chunks last
            for ki, (k, wk) in enumerate(kws):
                for j, tau in enumerate(taus):
                    nc.tensor.matmul(
                        pt[:, 512 * j:512 * (j + 1)],
                        wk[:, 128 * tau:128 * (tau + 1)],
                        s_sb[k],
                        start=(ki == 0),
                        stop=(ki == len(kws) - 1),
                    )

        for t in range(T):
            for l in range(2):
                w = w_sb[l]
                if t == 0 and l == 1:
                    kws = [(2, w1f_sb), (3, w[3]), (4, w[4])]
                elif l == 1:
                    # slot0 (h1) is old state, slot1 (h0) is being written:
                    # k2 needs h0 strip0, k3 strips 1-2, k4 strips 3-4
                    kws = [(k, w[k]) for k in (0, 1, 2, 3, 4)]
                else:
                    # slot1 (h0) is old, slot0 (h1) fresh: do k3,k4 first
                    kws = [(k, w[k]) for k in (3, 4, 0, 1, 2)]

                g_sb = gates_pool.tile([128, 5120], gate_dt, name="g", tag="g")
                o_bf = gates_pool.tile([128, 1536], BF16, name="ob", tag="ob")

                # ---- group A: tau0-3 = g gate (tanh) + f head (sigmoid) ----
                pt = psum_pool.tile([128, 2048], F32, name="psA", tag="ps")
                mm_group(pt, groups[0], kws)
                nc.scalar.activation(g_sb[:, 0:1024], pt[:, 0:1024], TANH)
                nc.scalar.activation(g_sb[0:64, 1024:1536], pt[0:64, 1024:1536], TANH)
                nc.scalar.activation(g_sb[64:128, 1024:1536], pt[64:128, 1024:1536], SIG)
                nc.scalar.activation(g_sb[:, 1536:2048], pt[:, 1536:2048], SIG)

                # ---- group B: tau4-7 = f tail, i, o head (all sigmoid) ----
                pt = psum_pool.tile([128, 2048], F32, name="psB", tag="ps")
                mm_group(pt, groups[1], kws)
                nc.scalar.activation(g_sb[:, 2048:2560], pt[:, 0:512], SIG)
                nc.scalar.activation(g_sb[:, 2560:3072], pt[:, 512:1024], SIG)
                nc.scalar.activation(g_sb[:, 3072:3584], pt[:, 1024:1536], SIG)
                nc.scalar.activation(g_sb[0:64, 3584:4096], pt[0:64, 1536:2048], SIG)
                nc.scalar.activation(o_bf[64:128, 0:512], pt[64:128, 1536:2048], SIG)

                # gate views. Hardware rule: a TensorTensor's two inputs must
                # share a base partition (the output may shift), so the
                # multiplied pairs are kept same-parity: g,i in the C layout
                # (strips/pieces at {128,256}), f,o in the F layout ({64,192}).
                g_s = (g_sb[0:64, 0:512], g_sb[64:128, 0:512],
                       g_sb[0:64, 512:1024], g_sb[64:128, 512:1024],
                       g_sb[0:64, 1024:1536])
                i_s = (g_sb[0:64, 2560:3072], g_sb[64:128, 2560:3072],
                       g_sb[0:64, 3072:3584], g_sb[64:128, 3072:3584],
                       g_sb[0:64, 3584:4096])
                f_r1, f_r2a, f_r2b = (g_sb[64:128, 1024:1536],
                                      g_sb[:, 1536:2048], g_sb[:, 2048:2560])
                o_s = (o_bf[64:128, 0:512], o_bf[0:64, 512:1024],
                       o_bf[64:128, 512:1024], o_bf[0:64, 1024:1536],
                       o_bf[64:128, 1024:1536])

                c = c_sb[l]
                t1 = cell_pool.tile([128, 1536], gate_dt, name="t1", tag="t1")
                t2 = cell_pool.tile([128, 1536], gate_dt, name="t2", tag="t2")
                # o and tanh(c) in bf16: the tail h muls hit the DVE 2x mode
                th = cell_pool.tile([128, 1536], BF16, name="th", tag="th")

                def f_strips(x):
                    return (x[64:128, 0:512], x[0:64, 512:1024],
                            x[64:128, 512:1024], x[0:64, 1024:1536],
                            x[64:128, 1024:1536])

                def f_pieces(x):
                    return (x[64:128, 0:512], x[:, 512:1024], x[:, 1024:1536])

                t1_s, t1_p = f_strips(t1), f_pieces(t1)
                c_p, t2_p = f_pieces(c), f_pieces(t2)
                th_p = f_pieces(th)
                th_s = f_strips(th)

                # cell chain, piecewise in F-layout pieces {r<64, 64-191,
                # 192-319}; t1 = i*g reads C-layout pairs, writes F (out
                # shift is legal), so it is split at {64,128,192,256}
                nc.vector.tensor_mul(t1_s[0], i_s[0], g_s[0])
                nc.vector.tensor_mul(t2_p[0], f_r1, c_p[0])
                nc.vector.tensor_add(c_p[0], t2_p[0], t1_p[0])
                nc.scalar.activation(th_p[0], c_p[0], TANH)

                nc.vector.tensor_mul(t1_s[1], i_s[1], g_s[1])
                nc.vector.tensor_mul(t1_s[2], i_s[2], g_s[2])
                nc.vector.tensor_mul(t2_p[1], f_r2a, c_p[1])
                nc.vector.tensor_add(c_p[1], t2_p[1], t1_p[1])
                nc.scalar.activation(th_p[1], c_p[1], TANH)

                nc.vector.tensor_mul(t1_s[3], i_s[3], g_s[3])
                nc.vector.tensor_mul(t1_s[4], i_s[4], g_s[4])
                nc.vector.tensor_mul(t2_p[2], f_r2b, c_p[2])
                nc.vector.tensor_add(c_p[2], t2_p[2], t1_p[2])
                nc.scalar.activation(th_p[2], c_p[2], TANH)

                # ---- group C: tau8-9 = o tail (sigmoid) ----
                pt = psum_pool.tile([128, 1024], F32, name="psC", tag="ps")
                mm_group(pt, groups[2], kws)
                nc.scalar.activation(o_bf[:, 512:1536], pt, SIG)

                # h = o * tanh(c); must follow ALL of this layer's matmuls
                # (it overwrites the state tiles the matmuls read).  Both
                # inputs are F-layout so h0 (64-shifted slot) needs no input
                # split: 3 ops; h1 (slot0) splits at the C-layout bounds.
                if l == 0:
                    nc.vector.tensor_mul(s_sb[2][64:128, :], o_s[0], th_s[0])
                    nc.vector.tensor_mul(s_sb[3][:, :], o_bf[:, 512:1024],
                                         th[:, 512:1024])
                    nc.vector.tensor_mul(s_sb[4][0:108, :],
                                         o_bf[0:108, 1024:1536],
                                         th[0:108, 1024:1536])
                else:
                    outs = (s_sb[0][0:64, :], s_sb[0][64:128, :],
                            s_sb[1][0:64, :], s_sb[1][64:128, :],
                            s_sb[2][0:44, :])
                    for si in range(5):
                        o_ap, th_ap = o_s[si], th_s[si]
                        if si == 4:  # real rows only; keeps the bias row
                            o_ap = o_ap[0:44, :]
                            th_ap = th_ap[0:44, :]
                        nc.vector.tensor_mul(outs[si], o_ap, th_ap)

            nc.sync.dma_start(out=yT[t, 0:128, :], in_=s_sb[0])
            nc.sync.dma_start(out=yT[t, 128:256, :], in_=s_sb[1])
            nc.sync.dma_start(out=yT[t, 256:300, :], in_=s_sb[2][0:44, :])

    return nc


_GATE_ORDER = (2, 1, 0, 3)  # ours [g,f,i,o] -> torch gate indices [i,f,g,o]


def _pack_w(w_x, w_h, b, swap):
    out = np.zeros((K, 1280), np.float32)
    for gi, og in enumerate(_GATE_ORDER):
        rows = slice(og * H, (og + 1) * H)
        cols = slice(gi * HP, gi * HP + H)
        out[0:H, cols] = w_x[rows, :].T
        out[300, cols] = b[rows]
        out[HP:HP + H, cols] = w_h[rows, :].T
    if swap:
        out = np.concatenate([out[HP:], out[:HP]], axis=0)
        out[300], out[HP + 300] = out[HP + 300].copy(), out[300].copy()
    return out.astype(ml_dtypes.bfloat16)


def _prep_shared(W_ih0, W_hh0, b0, W_ih1, W_hh1, b1):
    w0 = _pack_w(W_ih0, W_hh0, b0, swap=False)
    w1 = _pack_w(W_ih1, W_hh1, b1, swap=True)
    # chunk-2 rows 0-63 are slot0 (h1 side, zero at t=0 except bias row 44);
    # rows 64-127 are slot1 (h0 side) and must be kept
    w1f = np.array(w1[256:384], np.float32)
    w1f[0:44] = 0.0
    w1f[45:64] = 0.0
    return w0, w1, w1f.astype(ml_dtypes.bfloat16)


def prep_core_inputs(z_shard, W_ih0, W_hh0, b0, W_ih1, W_hh1, b1):
    """Single-core in_map (used by the dev/sim harnesses)."""
    w0, w1, w1f = _prep_shared(W_ih0, W_hh0, b0, W_ih1, W_hh1, b1)
    xz = np.zeros((HP, B), np.float32)
    xz[0:H, :] = z_shard.T
    xz[300, :] = 1.0
    return {"w0": w0, "w1": w1, "w1f": w1f,
            "xz": xz.astype(ml_dtypes.bfloat16)}


_NC_CACHE = {}
last_results = None


def kernel(z, W_ih0, W_hh0, b_ih0, b_hh0, W_ih1, W_hh1, b_ih1, b_hh1,
           sentence_len):
    global last_results
    from concourse.bass_utils import run_bass_kernel_spmd

    T = int(sentence_len)
    if T not in _NC_CACHE:
        nc = build_nc(T)
        nc.compile()
        _NC_CACHE[T] = nc
    nc = _NC_CACHE[T]

    z = np.asarray(z, np.float32)
    b0 = np.asarray(b_ih0, np.float32) + np.asarray(b_hh0, np.float32)
    b1 = np.asarray(b_ih1, np.float32) + np.asarray(b_hh1, np.float32)
    w0, w1, w1f = _prep_shared(np.asarray(W_ih0, np.float32),
                               np.asarray(W_hh0, np.float32), b0,
                               np.asarray(W_ih1, np.float32),
                               np.asarray(W_hh1, np.float32), b1)

    in_maps = []
    for i in range(N_CORES):
        xz = np.zeros((HP, B), np.float32)
        xz[0:H, :] = z[i * B:(i + 1) * B, :].T
        xz[300, :] = 1.0
        in_maps.append({"w0": w0, "w1": w1, "w1f": w1f,
                        "xz": xz.astype(ml_dtypes.bfloat16)})

    last_results = run_bass_kernel_spmd(
        nc, in_maps, core_ids=list(range(N_CORES)))

    out = np.empty((N_CORES * B, 1, T, H), np.float32)
    for i, r in enumerate(last_results.results):
        yT = np.asarray(r["yT"])  # [T, 300, 512] bf16
        u32 = yT.view(np.uint16).astype(np.uint32) << 16    # [T, H, B]
        out[i * B:(i + 1) * B, 0] = (
            u32.view(np.float32).transpose(2, 0, 1))
    return out
